# revision 1
# baseline (speedup 1.0000x reference)
"""Trainium2 Bass kernel for nn_Camada_33612414059004.

Computes, for x:[B,N,D,S], M:[N,N], w_syn:[N,D,S], b_dend:[N,D],
w_dend:[N,D], b_soma:[N]:

    xm    = einsum('bids,oi->bods', x, M)
    dend  = tanh(einsum('bnds,nds->bnd', xm, w_syn) + b_dend)
    soma  = einsum('bnd,nd->bn', dend, w_dend) + b_soma
    out   = sigmoid(soma)                                  # [B, N]

Sharding: data-parallel over batch across 8 NeuronCores (B=64 -> 8/core),
zero cross-core communication.  Per core the dominant work is the
connectivity matmul  M[o,i] @ x[i, (b,d,s)]  ([1024x1024]x[1024x1024],
2.15 GFLOP, bf16 operands / fp32 PSUM accumulate) on the TensorEngine.
The per-neuron stages run in fp32, spread so no engine exceeds the PE's
3.5us-per-o-tile matmul time: Vector multiplies the PSUM result by w_syn
and does the s/d reductions, GpSimd does the small bias-add and w_dend
multiply, Scalar does tanh / sigmoid(+b_soma).

Per-core on-chip layout: output neurons `o` on the 128 SBUF partitions
(8 o-tiles), free dim ordered (b, d, s).

Schedule: o-tiles 0-3 accumulate k-outer — per-k PE work (4 matmul pairs)
matches the per-k input DMA time, riding out the input stream; o-tiles
4-7 run k-inner one at a time, each tile's postprocess chain pipelining
against the next tile's matmuls.  x chunks load on the Sync HWDGE, mt
chunks on the Scalar HWDGE (parallel issue); all per-neuron parameters
are packed host-side into one contiguous [128, 1160] fp32 array moved by
a single DMA.
"""

import numpy as np
import ml_dtypes
from contextlib import ExitStack

import concourse.bass as bass
import concourse.mybir as mybir
import concourse.tile as tile

B, N, D, S = 64, 1024, 8, 16
NCORES = 8
BC = B // NCORES          # batches per core = 8
DS = D * S                # 128
P = 128                   # SBUF partitions
KT = N // P               # 8 contraction chunks (input neurons)
OT = N // P               # 8 output-neuron tiles
FH = 512                  # matmul moving free dim (one fp32 PSUM bank)
BD = BC * D               # 64
GRP = 4                   # o-tiles in the k-outer leading group
PCOLS = OT * DS + OT * D + OT * D + OT   # packed params: 1160

F32 = mybir.dt.float32
BF16 = mybir.dt.bfloat16

_NC_CACHE = {}


def legalize_waits(nc, max_attached=1):
    """Split multi-semaphore waits onto preceding same-engine NOPs.

    The walrus build in this environment accepts at most one sync-wait
    command per instruction (setupSyncWait: "Too many sync wait commands"),
    but Tile attaches one wait per out-of-date engine clock.  An engine is
    in-order, so hoisting the extra waits onto NOPs immediately before the
    instruction is semantics-preserving.
    """
    nid = 0
    for f in nc.m.functions:
        for blk in f.blocks:
            new = []
            changed = False
            for inst in blk.instructions:
                si = inst.sync_info
                if si is not None and si.on_wait and len(si.on_wait) > max_attached:
                    waits = list(si.on_wait)
                    for w in waits[:-max_attached]:
                        nid += 1
                        nop = mybir.InstNoOp(name=f"WSPLIT-{nid}", ins=[], outs=[])
                        nop.engine = inst.engine
                        nop.sync_info = mybir.SyncInfo(on_wait=[w], on_update=[])
                        new.append(nop)
                    inst.sync_info = mybir.SyncInfo(
                        on_wait=waits[-max_attached:], on_update=list(si.on_update)
                    )
                    changed = True
                new.append(inst)
            if changed:
                blk.instructions = new
    return nc


def build_nc(mm_dtype=BF16, legalize=True):
    """Build the single-core Bass program (SPMD: same program on all cores)."""
    nc = bass.Bass()
    mt = nc.declare_dram_parameter("mt", [N, N], mm_dtype, isOutput=False)
    xc = nc.declare_dram_parameter("xc", [N, BC * DS], mm_dtype, isOutput=False)
    params = nc.declare_dram_parameter("params", [P, PCOLS], F32, isOutput=False)
    out = nc.declare_dram_parameter("out", [P, OT * BC], F32, isOutput=True)

    AF = mybir.ActivationFunctionType
    AX = mybir.AxisListType
    OP = mybir.AluOpType

    with tile.TileContext(nc) as tc, ExitStack() as ctx:
        wpool = ctx.enter_context(tc.tile_pool(name="weights", bufs=1))
        xpool = ctx.enter_context(tc.tile_pool(name="xin", bufs=1))
        pspool = ctx.enter_context(tc.tile_pool(name="ps", bufs=8, space="PSUM"))
        prpool = ctx.enter_context(tc.tile_pool(name="prp", bufs=3))
        smpool = ctx.enter_context(tc.tile_pool(name="smp", bufs=3))

        # --- PE pre-warm: dummy matmuls on memset scratch while the first
        # input chunk is still in flight.  The HAM clock gate needs ~3.4us
        # of sustained PE activity to lift the PE from 1.2 to 2.4 GHz;
        # warming during the DMA wait means the real matmuls run at full
        # rate from the start.  Placed first so the memset precedes the
        # DMA issue on GpSimd and the dummies start right after the PE's
        # preamble. ---
        warm_sb = wpool.tile([P, FH], BF16, tag="warm", name="warm_sb")
        nc.gpsimd.memset(warm_sb[:], 0.0)
        warm_ps = pspool.tile([P, FH], F32, tag="ps", name="warm_ps")
        for _ in range(8):
            nc.tensor.matmul(
                warm_ps[:], lhsT=warm_sb[:, 0:P], rhs=warm_sb[:],
                start=True, stop=True,
            )

        # --- input DMAs: x chunks on Sync, mt chunks on Scalar (parallel
        # HWDGE issue); whole [128, 2KB-row] chunks for full DMA rate. ---
        x_tiles, mt_tiles = [], []
        x0_dma = None
        for k in range(KT):
            xt = xpool.tile([P, BC * DS], mm_dtype, tag=f"x{k}", name=f"x{k}")
            mtk = xpool.tile([P, N], mm_dtype, tag=f"m{k}", name=f"m{k}")
            xdma = nc.sync.dma_start(xt[:], xc[k * P:(k + 1) * P, :])
            if k == 0:
                x0_dma = xdma
            nc.scalar.dma_start(mtk[:], mt[k * P:(k + 1) * P, :])
            x_tiles.append(xt)
            mt_tiles.append(mtk)

        # Delay the (non-critical) params transfer behind the first x chunk
        # so it doesn't steal HBM bandwidth from the matmul-critical loads.
        params_sb = wpool.tile([P, PCOLS], F32, tag="params", name="params_sb")
        pdma = nc.gpsimd.dma_start(params_sb[:], params[:, :])
        from bass_rust import add_dep_helper
        add_dep_helper(pdma.ins, x0_dma.ins, sync=True,
                       reason="params after critical first chunk")
        W0, B0, W1, B1 = 0, OT * DS, OT * DS + OT * D, OT * DS + 2 * OT * D

        out_sb = wpool.tile([P, OT * BC], F32, tag="out", name="out_sb")

        def postprocess(t, pst, latency_opt=False, gps_heavy=False):
            # prod[o, b, (d,s)] = xm * w_syn (broadcast over b), read
            # straight from PSUM per half (fp32, 1x DVE).
            prod = prpool.tile([P, BC * DS], F32, tag="prod", name=f"prod{t}")
            for h in range(2):
                nc.vector.tensor_mul(
                    prod[:, h * FH:(h + 1) * FH].rearrange(
                        "p (b q) -> p b q", b=BC // 2),
                    pst[h][:].rearrange("p (b q) -> p b q", b=BC // 2),
                    params_sb[:, W0 + t * DS:W0 + (t + 1) * DS].unsqueeze(1)
                    .broadcast_to([P, BC // 2, DS]),
                )
            # Big s-reduce stays on DVE, contiguous with the mults so the
            # in-order DVE stream never stalls on another engine mid-tile.
            dp = smpool.tile([P, BD], F32, tag="dp", name=f"dp{t}")
            pv = prod[:].rearrange("p (bd s) -> p bd s", s=S)
            if gps_heavy:
                # Whole s-reduce as a GpSimd pairwise tree: frees the
                # in-order DVE right after the PSUM mults so the LAST
                # tile's latency chain is not blocked behind this one.
                gr1 = smpool.tile([P, BD * 8], F32, tag="gr1", name=f"gr1{t}")
                nc.gpsimd.tensor_add(
                    gr1[:].rearrange("p (bd s) -> p bd s", s=8),
                    pv[:, :, 0:8], pv[:, :, 8:16],
                )
                g1v = gr1[:].rearrange("p (bd s) -> p bd s", s=8)
                gr2 = smpool.tile([P, BD * 4], F32, tag="gr2", name=f"gr2{t}")
                nc.gpsimd.tensor_add(
                    gr2[:].rearrange("p (bd s) -> p bd s", s=4),
                    g1v[:, :, 0:4], g1v[:, :, 4:8],
                )
                g2v = gr2[:].rearrange("p (bd s) -> p bd s", s=4)
                gr3 = smpool.tile([P, BD * 2], F32, tag="gr3", name=f"gr3{t}")
                nc.gpsimd.tensor_add(
                    gr3[:].rearrange("p (bd s) -> p bd s", s=2),
                    g2v[:, :, 0:2], g2v[:, :, 2:4],
                )
                g3v = gr3[:].rearrange("p (bd s) -> p bd s", s=2)
                nc.gpsimd.tensor_add(
                    dp[:].unsqueeze(2), g3v[:, :, 0:1], g3v[:, :, 1:2],
                )
            else:
                nc.vector.tensor_reduce(dp[:], pv, axis=AX.X, op=OP.add)
            bias_eng = nc.vector if latency_opt else nc.gpsimd
            bias_eng.tensor_add(
                dp[:].rearrange("p (b d) -> p b d", d=D),
                dp[:].rearrange("p (b d) -> p b d", d=D),
                params_sb[:, B0 + t * D:B0 + (t + 1) * D].unsqueeze(1)
                .broadcast_to([P, BC, D]),
            )
            dend = smpool.tile([P, BD], F32, tag="dend", name=f"dend{t}")
            nc.scalar.activation(dend[:], dp[:], AF.Tanh)
            # soma: * w_dend, reduce over d, sigmoid(+b_soma)
            sp = smpool.tile([P, BD], F32, tag="sp", name=f"sp{t}")
            soma = smpool.tile([P, BC], F32, tag="soma", name=f"soma{t}")
            if latency_opt:
                nc.vector.tensor_mul(
                    sp[:].rearrange("p (b d) -> p b d", d=D),
                    dend[:].rearrange("p (b d) -> p b d", d=D),
                    params_sb[:, W1 + t * D:W1 + (t + 1) * D].unsqueeze(1)
                    .broadcast_to([P, BC, D]),
                )
                nc.vector.tensor_reduce(
                    soma[:],
                    sp[:].rearrange("p (b d) -> p b d", d=D),
                    axis=AX.X,
                    op=OP.add,
                )
            else:
                # Soma stage entirely on GpSimd (mult + pairwise d-tree),
                # keeping the DVE stream free for the next tile's mults.
                nc.gpsimd.tensor_mul(
                    sp[:].rearrange("p (b d) -> p b d", d=D),
                    dend[:].rearrange("p (b d) -> p b d", d=D),
                    params_sb[:, W1 + t * D:W1 + (t + 1) * D].unsqueeze(1)
                    .broadcast_to([P, BC, D]),
                )
                r1 = smpool.tile([P, BC * 4], F32, tag="r1", name=f"r1{t}")
                spv = sp[:].rearrange("p (b d) -> p b d", d=D)
                nc.gpsimd.tensor_add(
                    r1[:].rearrange("p (b d) -> p b d", d=4),
                    spv[:, :, 0:4], spv[:, :, 4:8],
                )
                r2 = smpool.tile([P, BC * 2], F32, tag="r2", name=f"r2{t}")
                r1v = r1[:].rearrange("p (b d) -> p b d", d=4)
                nc.gpsimd.tensor_add(
                    r2[:].rearrange("p (b d) -> p b d", d=2),
                    r1v[:, :, 0:2], r1v[:, :, 2:4],
                )
                r2v = r2[:].rearrange("p (b d) -> p b d", d=2)
                nc.gpsimd.tensor_add(
                    soma[:].unsqueeze(2), r2v[:, :, 0:1], r2v[:, :, 1:2],
                )
            nc.scalar.activation(
                out_sb[:, t * BC:(t + 1) * BC], soma[:], AF.Sigmoid,
                bias=params_sb[:, B1 + t:B1 + t + 1],
            )

        def mm(pst, t, k):
            for h in range(2):
                nc.tensor.matmul(
                    pst[h][:],
                    lhsT=mt_tiles[k][:, t * P:(t + 1) * P],
                    rhs=x_tiles[k][:, h * FH:(h + 1) * FH],
                    start=(k == 0),
                    stop=(k == KT - 1),
                )

        # Leading group: k-outer over o-tiles 0..GRP-1 — per-k PE work
        # (GRP matmul pairs) paces with the per-k chunk DMA.
        pst = {}
        for t in range(GRP):
            pst[t] = [
                pspool.tile([P, FH], F32, tag="ps", name=f"ps{t}_{h}")
                for h in range(2)
            ]
        for k in range(KT):
            for t in range(GRP):
                mm(pst[t], t, k)
        for t in range(GRP):
            postprocess(t, pst[t])

        # Remaining o-tiles: one at a time, k-inner; each tile's chain
        # overlaps the next tile's matmuls.
        for t in range(GRP, OT):
            pstt = [
                pspool.tile([P, FH], F32, tag="ps", name=f"ps{t}_{h}")
                for h in range(2)
            ]
            for k in range(KT):
                mm(pstt, t, k)
            postprocess(t, pstt, latency_opt=(t == OT - 1))

        nc.scalar.dma_start(out[:, :], out_sb[:])

    if legalize:
        legalize_waits(nc)
    return nc


def get_nc(mm_dtype=BF16):
    key = str(mm_dtype)
    if key not in _NC_CACHE:
        _NC_CACHE[key] = build_nc(mm_dtype)
    return _NC_CACHE[key]


def pack_params(w_syn, b_dend, w_dend, b_soma):
    """Pack per-neuron parameters into one [128, 1160] fp32 array whose
    columns match the SBUF params tile layout (w_syn | b_dend | w_dend |
    b_soma, each o-tile-major)."""
    ws = np.asarray(w_syn, np.float32).reshape(OT, P, DS).transpose(1, 0, 2).reshape(P, OT * DS)
    bd = np.asarray(b_dend, np.float32).reshape(OT, P, D).transpose(1, 0, 2).reshape(P, OT * D)
    wd = np.asarray(w_dend, np.float32).reshape(OT, P, D).transpose(1, 0, 2).reshape(P, OT * D)
    bs = np.asarray(b_soma, np.float32).reshape(OT, P).T
    return np.ascontiguousarray(np.concatenate([ws, bd, wd, bs], axis=1))


def prepare_in_maps(x, matriz_conexao, w_syn, b_dend, w_dend, b_soma,
                    mm_np_dtype=ml_dtypes.bfloat16):
    x = np.asarray(x, dtype=np.float32)
    mt_np = np.ascontiguousarray(np.asarray(matriz_conexao, np.float32).T).astype(mm_np_dtype)
    params_np = pack_params(w_syn, b_dend, w_dend, b_soma)
    xt = np.ascontiguousarray(x.transpose(1, 0, 2, 3).reshape(N, B, DS))
    in_maps = []
    for c in range(NCORES):
        xc_np = np.ascontiguousarray(
            xt[:, c * BC:(c + 1) * BC, :].reshape(N, BC * DS)
        ).astype(mm_np_dtype)
        in_maps.append({"mt": mt_np, "xc": xc_np, "params": params_np})
    return in_maps


def assemble_output(results):
    outs = []
    for c in range(NCORES):
        oc = np.asarray(results[c]["out"])          # [P, OT*BC] = (oi, (t, b))
        outs.append(oc.reshape(P, OT, BC).transpose(2, 1, 0).reshape(BC, N))
    return np.ascontiguousarray(np.concatenate(outs, axis=0).astype(np.float32))


def kernel(x, matriz_conexao, w_syn, b_dend, w_dend, b_soma):
    from concourse.bass_utils import run_bass_kernel_spmd
    in_maps = prepare_in_maps(x, matriz_conexao, w_syn, b_dend, w_dend, b_soma)
    nc = get_nc()
    res = run_bass_kernel_spmd(nc, in_maps, list(range(NCORES)))
    return assemble_output(res.results)



# revision 3
# speedup vs baseline: 1.0659x; 1.0659x over previous
"""Trainium2 Bass kernel for nn_Camada_33612414059004.

Computes, for x:[B,N,D,S], M:[N,N], w_syn:[N,D,S], b_dend:[N,D],
w_dend:[N,D], b_soma:[N]:

    xm    = einsum('bids,oi->bods', x, M)
    dend  = tanh(einsum('bnds,nds->bnd', xm, w_syn) + b_dend)
    soma  = einsum('bnd,nd->bn', dend, w_dend) + b_soma
    out   = sigmoid(soma)                                  # [B, N]

Sharding: data-parallel over batch across 8 NeuronCores (B=64 -> 8/core),
zero cross-core communication.  Per core the dominant work is the
connectivity matmul  M[o,i] @ x[i, (b,d,s)]  ([1024x1024]x[1024x1024]).

Key speed tricks vs the bf16 baseline:
 - fp8(e4m3) matmul operands with perf_mode=DoubleRow: M is an exact 0/1
   matrix (fp8-lossless) and x quantization costs ~0.5% final rel-err
   (tanh/sigmoid compress it).  DoubleRow packs 2 fp8 weights per PE
   cell -> 256-deep contraction per matmul, ~2x PE throughput, and the
   input DMA bytes halve (2MB -> 1MB per tensor per core).
 - 4 contraction pair-steps of 256 input-neurons; per o-tile a single
   [128, 1024] fp32 PSUM tile (2 banks) accumulates all 4 steps, and is
   drained by ONE wide op instead of two per-half ops.
 - Postprocess split across engines so no engine exceeds the PE pace:
   PSUM drain+w_syn-multiply on DVE for even tiles, on ACT(copy)+GpSimd
   (multiply) for odd tiles; s-reduce on DVE (2x all-SBUF mode) or a
   GpSimd pairwise tree; tanh/sigmoid batched across tile groups on ACT
   with b_soma folded into the soma reduce (scalar_tensor_tensor).
 - Last o-tile runs its two PSUM halves (= batch halves) as independent
   latency-optimized all-DVE chains, h0 postprocess overlapping h1
   matmuls.
"""

import numpy as np
import ml_dtypes
from contextlib import ExitStack

import concourse.bass as bass
import concourse.mybir as mybir
import concourse.tile as tile

B, N, D, S = 64, 1024, 8, 16
NCORES = 8
BC = B // NCORES          # batches per core = 8
DS = D * S                # 128
P = 128                   # SBUF partitions
KP = 4                    # contraction pair-steps (256 input neurons each)
OT = N // P               # 8 output-neuron tiles
FH = 512                  # matmul moving free dim (one fp32 PSUM bank)
BD = BC * D               # 64
GRP = 4                   # o-tiles in the k-outer leading group
B0, W1, B1 = 0, OT * D, 2 * OT * D
SPC = 2 * OT * D + OT     # small params cols = 136

F32 = mybir.dt.float32
BF16 = mybir.dt.bfloat16
F8 = mybir.dt.float8e4

_NC_CACHE = {}


def legalize_waits(nc, max_attached=1):
    """Split multi-semaphore waits onto preceding same-engine NOPs.

    The walrus build in this environment accepts at most one sync-wait
    command per instruction (setupSyncWait: "Too many sync wait commands"),
    but Tile attaches one wait per out-of-date engine clock.  An engine is
    in-order, so hoisting the extra waits onto NOPs immediately before the
    instruction is semantics-preserving.
    """
    nid = 0
    for f in nc.m.functions:
        for blk in f.blocks:
            new = []
            changed = False
            for inst in blk.instructions:
                si = inst.sync_info
                if si is not None and si.on_wait and len(si.on_wait) > max_attached:
                    waits = list(si.on_wait)
                    for w in waits[:-max_attached]:
                        nid += 1
                        nop = mybir.InstNoOp(name=f"WSPLIT-{nid}", ins=[], outs=[])
                        nop.engine = inst.engine
                        nop.sync_info = mybir.SyncInfo(on_wait=[w], on_update=[])
                        new.append(nop)
                    inst.sync_info = mybir.SyncInfo(
                        on_wait=waits[-max_attached:], on_update=list(si.on_update)
                    )
                    changed = True
                new.append(inst)
            if changed:
                blk.instructions = new
    return nc


def build_nc(legalize=True):
    """Build the single-core Bass program (SPMD: same program on all cores)."""
    nc = bass.Bass()
    # mt cols: (o-tile t, pair-member j, o-within-tile) so per-o-tile lhsT
    # slices and the tile-0-first DMA split are both contiguous.
    mt = nc.declare_dram_parameter("mt", [KP * P, OT * 2 * P], F8, isOutput=False)
    # xc cols: (half h, pair-member j, (b%4, d, s)) so per-half rhs slices
    # and the half-0-first DMA split are both contiguous.
    xc = nc.declare_dram_parameter("xc", [KP * P, 2 * 2 * FH], F8, isOutput=False)
    wsyn = nc.declare_dram_parameter("wsyn", [P, OT * DS], BF16, isOutput=False)
    smallp = nc.declare_dram_parameter("smallp", [P, SPC], F32, isOutput=False)
    out = nc.declare_dram_parameter("out", [P, OT * BC], F32, isOutput=True)

    AF = mybir.ActivationFunctionType
    AX = mybir.AxisListType
    OP = mybir.AluOpType
    DR = mybir.MatmulPerfMode.DoubleRow

    with tile.TileContext(nc) as tc, ExitStack() as ctx:
        wpool = ctx.enter_context(tc.tile_pool(name="weights", bufs=1))
        xpool = ctx.enter_context(tc.tile_pool(name="xin", bufs=1))
        pspool = ctx.enter_context(tc.tile_pool(name="ps", bufs=4, space="PSUM"))
        prpool = ctx.enter_context(tc.tile_pool(name="prp", bufs=3))
        cpool = ctx.enter_context(tc.tile_pool(name="cpp", bufs=2))
        smpool = ctx.enter_context(tc.tile_pool(name="smp", bufs=3))

        # --- PE pre-warm: dummy matmuls on memset scratch while the first
        # input chunk is in flight, so the HAM clock-gate 3.4us activity
        # window starts as early as possible. ---
        warm_sb = wpool.tile([P, FH], BF16, tag="warm", name="warm_sb")
        nc.gpsimd.memset(warm_sb[:], 0.0)
        warm_ps = pspool.tile([P, 2 * FH], F32, tag="ps", name="warm_ps")
        for _ in range(3):
            nc.tensor.matmul(
                warm_ps[:, 0:FH], lhsT=warm_sb[:, 0:P], rhs=warm_sb[:],
                start=True, stop=True,
            )

        # --- input DMAs: x on Sync HWDGE, mt on Scalar HWDGE (parallel
        # issue).  Pair 0 is split so the very first matmul (o-tile 0,
        # half 0) starts as soon as ~160KB has landed. ---
        x_tiles, mt_tiles = [], []
        xdmas = []
        for p in range(KP):
            xt = xpool.tile([P, 2 * 2 * FH], F8, tag=f"x{p}", name=f"x{p}")
            mtp = xpool.tile([P, OT * 2 * P], F8, tag=f"m{p}", name=f"m{p}")
            r0, r1 = p * P, (p + 1) * P
            if p == 0:
                xdmas.append(nc.sync.dma_start(xt[:, 0:2 * FH], xc[r0:r1, 0:2 * FH]))
                xdmas.append(
                    nc.sync.dma_start(xt[:, 2 * FH:4 * FH], xc[r0:r1, 2 * FH:4 * FH]))
                nc.scalar.dma_start(mtp[:, 0:2 * P], mt[r0:r1, 0:2 * P])
                nc.scalar.dma_start(mtp[:, 2 * P:], mt[r0:r1, 2 * P:])
            else:
                xdmas.append(nc.sync.dma_start(xt[:], xc[r0:r1, :]))
                nc.scalar.dma_start(mtp[:], mt[r0:r1, :])
            x_tiles.append(xt)
            mt_tiles.append(mtp)

        # Per-neuron params ride behind the matmul-critical stream: they are
        # first needed when the leading postprocess starts (~after pair 3).
        from bass_rust import add_dep_helper
        smallp_sb = wpool.tile([P, SPC], F32, tag="smallp", name="smallp_sb")
        sdma = nc.gpsimd.dma_start(smallp_sb[:], smallp[:, :])
        add_dep_helper(sdma.ins, xdmas[1].ins, sync=True,
                       reason="small params after first x pair")
        wsyn_sb = wpool.tile([P, OT * DS], BF16, tag="wsyn", name="wsyn_sb")
        wdma = nc.gpsimd.dma_start(wsyn_sb[:], wsyn[:, :])
        add_dep_helper(wdma.ins, xdmas[2].ins, sync=True,
                       reason="w_syn after second x pair")

        dp_all = wpool.tile([P, OT * BD], F32, tag="dp", name="dp_all")
        dend_all = wpool.tile([P, OT * BD], F32, tag="dend", name="dend_all")
        soma_all = wpool.tile([P, OT * BC], F32, tag="soma", name="soma_all")
        out_sb = wpool.tile([P, OT * BC], F32, tag="out", name="out_sb")

        def mm(pst, t, p, h):
            nc.tensor.matmul(
                pst[:, h * FH:(h + 1) * FH],
                lhsT=mt_tiles[p][:, t * 2 * P:(t + 1) * 2 * P]
                .rearrange("p (j o) -> p j o", j=2),
                rhs=x_tiles[p][:, h * 2 * FH:(h + 1) * 2 * FH]
                .rearrange("p (j c) -> p j c", j=2),
                start=(p == 0), stop=(p == KP - 1),
                perf_mode=DR,
            )

        def wsyn_bc(t, b):
            return (wsyn_sb[:, t * DS:(t + 1) * DS].unsqueeze(1)
                    .broadcast_to([P, b, DS]))

        def drain_dve(t, pst):
            # prod[o, b, (d,s)] = psum * w_syn (broadcast over b), one wide
            # DVE op straight from the 2-bank PSUM tile.
            prod = prpool.tile([P, BC * DS], F32, tag="prod", name=f"prod{t}")
            nc.vector.tensor_mul(
                prod[:].rearrange("p (b q) -> p b q", b=BC),
                pst[:].rearrange("p (b q) -> p b q", b=BC),
                wsyn_bc(t, BC),
            )
            return prod

        def drain_act(t, pst):
            # ACT copies PSUM->SBUF (only DVE/ACT can read PSUM); GpSimd
            # does the w_syn multiply from SBUF.  Keeps DVE lean.
            cpy = cpool.tile([P, BC * DS], F32, tag="cp", name=f"cp{t}")
            nc.scalar.activation(cpy[:], pst[:], AF.Copy)
            prod = prpool.tile([P, BC * DS], F32, tag="prod", name=f"prod{t}")
            nc.gpsimd.tensor_mul(
                prod[:].rearrange("p (b q) -> p b q", b=BC),
                cpy[:].rearrange("p (b q) -> p b q", b=BC),
                wsyn_bc(t, BC),
            )
            return prod

        def dp_slice(t):
            return dp_all[:, t * BD:(t + 1) * BD]

        def sred_dve(t, prod):
            nc.vector.tensor_reduce(
                dp_slice(t),
                prod[:].rearrange("p (bd s) -> p bd s", s=S),
                axis=AX.X, op=OP.add,
            )

        def sred_gps(t, prod):
            # GpSimd pairwise tree (no free-dim tensor_reduce on Q7).
            pv = prod[:].rearrange("p (bd s) -> p bd s", s=S)
            gr1 = smpool.tile([P, BD * 8], F32, tag="gr1", name=f"gr1{t}")
            nc.gpsimd.tensor_add(
                gr1[:].rearrange("p (bd s) -> p bd s", s=8),
                pv[:, :, 0:8], pv[:, :, 8:16])
            g1v = gr1[:].rearrange("p (bd s) -> p bd s", s=8)
            gr2 = smpool.tile([P, BD * 4], F32, tag="gr2", name=f"gr2{t}")
            nc.gpsimd.tensor_add(
                gr2[:].rearrange("p (bd s) -> p bd s", s=4),
                g1v[:, :, 0:4], g1v[:, :, 4:8])
            g2v = gr2[:].rearrange("p (bd s) -> p bd s", s=4)
            gr3 = smpool.tile([P, BD * 2], F32, tag="gr3", name=f"gr3{t}")
            nc.gpsimd.tensor_add(
                gr3[:].rearrange("p (bd s) -> p bd s", s=2),
                g2v[:, :, 0:2], g2v[:, :, 2:4])
            g3v = gr3[:].rearrange("p (bd s) -> p bd s", s=2)
            nc.gpsimd.tensor_add(
                dp_slice(t).unsqueeze(2), g3v[:, :, 0:1], g3v[:, :, 1:2])

        def bias_gps(t):
            nc.gpsimd.tensor_add(
                dp_slice(t).rearrange("p (b d) -> p b d", d=D),
                dp_slice(t).rearrange("p (b d) -> p b d", d=D),
                smallp_sb[:, B0 + t * D:B0 + (t + 1) * D].unsqueeze(1)
                .broadcast_to([P, BC, D]),
            )

        def tanh_batch(t_lo, t_hi):
            nc.scalar.activation(
                dend_all[:, t_lo * BD:t_hi * BD],
                dp_all[:, t_lo * BD:t_hi * BD], AF.Tanh)

        def soma_dve(t):
            sp = smpool.tile([P, BD], F32, tag="sp", name=f"sp{t}")
            nc.vector.tensor_mul(
                sp[:].rearrange("p (b d) -> p b d", d=D),
                dend_all[:, t * BD:(t + 1) * BD]
                .rearrange("p (b d) -> p b d", d=D),
                smallp_sb[:, W1 + t * D:W1 + (t + 1) * D].unsqueeze(1)
                .broadcast_to([P, BC, D]),
            )
            smp = smpool.tile([P, BC], F32, tag="smp", name=f"smp{t}")
            nc.vector.tensor_reduce(
                smp[:], sp[:].rearrange("p (b d) -> p b d", d=D),
                axis=AX.X, op=OP.add)
            nc.vector.tensor_add(
                soma_all[:, t * BC:(t + 1) * BC], smp[:],
                smallp_sb[:, B1 + t:B1 + t + 1].broadcast_to([P, BC]))

        def soma_gps(t):
            sp = smpool.tile([P, BD], F32, tag="sp", name=f"sp{t}")
            nc.gpsimd.tensor_mul(
                sp[:].rearrange("p (b d) -> p b d", d=D),
                dend_all[:, t * BD:(t + 1) * BD]
                .rearrange("p (b d) -> p b d", d=D),
                smallp_sb[:, W1 + t * D:W1 + (t + 1) * D].unsqueeze(1)
                .broadcast_to([P, BC, D]),
            )
            spv = sp[:].rearrange("p (b d) -> p b d", d=D)
            r1 = smpool.tile([P, BC * 4], F32, tag="r1", name=f"r1{t}")
            nc.gpsimd.tensor_add(
                r1[:].rearrange("p (b d) -> p b d", d=4),
                spv[:, :, 0:4], spv[:, :, 4:8])
            r1v = r1[:].rearrange("p (b d) -> p b d", d=4)
            r2 = smpool.tile([P, BC * 2], F32, tag="r2", name=f"r2{t}")
            nc.gpsimd.tensor_add(
                r2[:].rearrange("p (b d) -> p b d", d=2),
                r1v[:, :, 0:2], r1v[:, :, 2:4])
            r2v = r2[:].rearrange("p (b d) -> p b d", d=2)
            smg = smpool.tile([P, BC], F32, tag="smp", name=f"smg{t}")
            nc.gpsimd.tensor_add(
                smg[:].unsqueeze(2), r2v[:, :, 0:1], r2v[:, :, 1:2])
            # b_soma folded here so the sigmoid can batch tiles with
            # different b_soma columns.
            nc.gpsimd.tensor_add(
                soma_all[:, t * BC:(t + 1) * BC], smg[:],
                smallp_sb[:, B1 + t:B1 + t + 1].broadcast_to([P, BC]))

        def sigmoid_batch(t_lo, t_hi):
            nc.scalar.activation(
                out_sb[:, t_lo * BC:t_hi * BC],
                soma_all[:, t_lo * BC:t_hi * BC], AF.Sigmoid)

        # ---- matmuls ----
        pst = {}
        for t in range(GRP):
            pst[t] = pspool.tile([P, 2 * FH], F32, tag="ps", name=f"ps{t}")
        # Leading group: pair-outer over o-tiles 0-3 rides the DMA stream.
        for p in range(KP):
            for t in range(GRP):
                mm(pst[t], t, p, 0)
                mm(pst[t], t, p, 1)
        # Trailing o-tiles: pair-inner, one at a time; each tile's
        # postprocess overlaps the next tile's matmuls.
        for t in range(GRP, OT):
            pst[t] = pspool.tile([P, 2 * FH], F32, tag="ps", name=f"ps{t}")
            if t < OT - 1:
                for p in range(KP):
                    mm(pst[t], t, p, 0)
                    mm(pst[t], t, p, 1)
            else:
                # Last tile half-outer: h0's chain overlaps h1's matmuls.
                for h in range(2):
                    for p in range(KP):
                        mm(pst[t], t, p, h)

        # ---- postprocess ----
        # Leading tiles 0-3 (stops cluster just after pair 3 lands):
        # drains alternate DVE / ACT+GpSimd; batched tanh+sigmoid.
        p0 = drain_dve(0, pst[0]); sred_dve(0, p0)
        p1 = drain_act(1, pst[1]); sred_gps(1, p1)
        p2 = drain_dve(2, pst[2]); sred_dve(2, p2)
        p3 = drain_act(3, pst[3]); sred_gps(3, p3)
        for t in range(4):
            bias_gps(t)
        tanh_batch(0, 4)
        soma_dve(0)
        soma_gps(1)
        soma_dve(2)
        soma_gps(3)
        sigmoid_batch(0, 4)
        # Trailing tiles 4-6.
        p4 = drain_dve(4, pst[4]); sred_dve(4, p4)
        p5 = drain_act(5, pst[5]); sred_gps(5, p5)
        bias_gps(4)
        bias_gps(5)
        tanh_batch(4, 6)
        soma_dve(4)
        soma_gps(5)
        sigmoid_batch(4, 6)
        nc.scalar.dma_start(out[:, 0:6 * BC], out_sb[:, 0:6 * BC])
        p6 = drain_dve(6, pst[6]); sred_dve(6, p6)
        bias_gps(6)
        tanh_batch(6, 7)
        soma_gps(6)
        sigmoid_batch(6, 7)
        # Last tile: two independent all-DVE latency chains (one per
        # PSUM half = batch half).
        t = OT - 1
        HB = BC // 2          # 4 batches per half
        for h in range(2):
            prod7 = prpool.tile([P, HB * DS], F32, tag="prod", name=f"prod7{h}")
            nc.vector.tensor_mul(
                prod7[:].rearrange("p (b q) -> p b q", b=HB),
                pst[t][:, h * FH:(h + 1) * FH]
                .rearrange("p (b q) -> p b q", b=HB),
                wsyn_bc(t, HB),
            )
            dps = dp_all[:, t * BD + h * HB * D:t * BD + (h + 1) * HB * D]
            nc.vector.tensor_reduce(
                dps, prod7[:].rearrange("p (bd s) -> p bd s", s=S),
                axis=AX.X, op=OP.add)
            nc.vector.tensor_add(
                dps.rearrange("p (b d) -> p b d", d=D),
                dps.rearrange("p (b d) -> p b d", d=D),
                smallp_sb[:, B0 + t * D:B0 + (t + 1) * D].unsqueeze(1)
                .broadcast_to([P, HB, D]),
            )
            dnds = dend_all[:, t * BD + h * HB * D:t * BD + (h + 1) * HB * D]
            nc.scalar.activation(
                dnds, dps, AF.Tanh)
            sp7 = smpool.tile([P, HB * D], F32, tag="sp", name=f"sp7{h}")
            nc.vector.tensor_mul(
                sp7[:].rearrange("p (b d) -> p b d", d=D),
                dnds.rearrange("p (b d) -> p b d", d=D),
                smallp_sb[:, W1 + t * D:W1 + (t + 1) * D].unsqueeze(1)
                .broadcast_to([P, HB, D]),
            )
            smp7 = smpool.tile([P, HB], F32, tag="smp", name=f"smp7{h}")
            nc.vector.tensor_reduce(
                smp7[:], sp7[:].rearrange("p (b d) -> p b d", d=D),
                axis=AX.X, op=OP.add)
            sms = soma_all[:, t * BC + h * HB:t * BC + (h + 1) * HB]
            nc.vector.tensor_add(
                sms, smp7[:],
                smallp_sb[:, B1 + t:B1 + t + 1].broadcast_to([P, HB]))
            nc.scalar.activation(
                out_sb[:, t * BC + h * HB:t * BC + (h + 1) * HB],
                sms, AF.Sigmoid)

        nc.scalar.dma_start(out[:, 6 * BC:], out_sb[:, 6 * BC:])

    if legalize:
        legalize_waits(nc)
    return nc


def get_nc():
    if "nc" not in _NC_CACHE:
        _NC_CACHE["nc"] = build_nc()
    return _NC_CACHE["nc"]


def pack_static(matriz_conexao, w_syn, b_dend, w_dend, b_soma):
    """Pack the batch-independent operands (shared by all cores)."""
    # mt rows (pair p, r), cols (t, j, o): lhsT[r, j, o] = M[t*128+o, (2p+j)*128+r]
    mtT = np.ascontiguousarray(np.asarray(matriz_conexao, np.float32).T)  # [i, o]
    mt_np = (mtT.reshape(KP, 2, P, OT, P)        # [p, j, r, t, o]
             .transpose(0, 2, 3, 1, 4)           # [p, r, t, j, o]
             .reshape(KP * P, OT * 2 * P)
             .astype(ml_dtypes.float8_e4m3))
    ws = (np.asarray(w_syn, np.float32).reshape(OT, P, DS).transpose(1, 0, 2)
          .reshape(P, OT * DS).astype(ml_dtypes.bfloat16))
    bd = np.asarray(b_dend, np.float32).reshape(OT, P, D).transpose(1, 0, 2).reshape(P, OT * D)
    wd = np.asarray(w_dend, np.float32).reshape(OT, P, D).transpose(1, 0, 2).reshape(P, OT * D)
    bs = np.asarray(b_soma, np.float32).reshape(OT, P).T
    smallp_np = np.ascontiguousarray(
        np.concatenate([bd, wd, bs], axis=1).astype(np.float32))
    return mt_np, np.ascontiguousarray(ws), smallp_np


def prepare_in_maps(x, matriz_conexao, w_syn, b_dend, w_dend, b_soma):
    mt_np, ws_np, smallp_np = pack_static(matriz_conexao, w_syn, b_dend, w_dend, b_soma)
    x = np.asarray(x, np.float32)
    xq = x.astype(ml_dtypes.float8_e4m3)
    # xt[i, b, (d,s)] then per core rows (p, r), cols (h, j, b4, d, s)
    xt = np.ascontiguousarray(xq.transpose(1, 0, 2, 3).reshape(N, B, DS))
    in_maps = []
    for c in range(NCORES):
        xcor = xt[:, c * BC:(c + 1) * BC, :]          # [N, 8, 128]
        xc_np = np.ascontiguousarray(
            xcor.reshape(KP, 2, P, 2, BC // 2, DS)     # [p, j, r, h, b4, ds]
            .transpose(0, 2, 3, 1, 4, 5)               # [p, r, h, j, b4, ds]
            .reshape(KP * P, 2 * 2 * FH))
        in_maps.append({"mt": mt_np, "xc": xc_np,
                        "wsyn": ws_np, "smallp": smallp_np})
    return in_maps


def assemble_output(results):
    outs = []
    for c in range(NCORES):
        oc = np.asarray(results[c]["out"])          # [P, (t, b)]
        outs.append(oc.reshape(P, OT, BC).transpose(2, 1, 0).reshape(BC, N))
    return np.ascontiguousarray(np.concatenate(outs, axis=0).astype(np.float32))


def kernel(x, matriz_conexao, w_syn, b_dend, w_dend, b_soma):
    from concourse.bass_utils import run_bass_kernel_spmd
    in_maps = prepare_in_maps(x, matriz_conexao, w_syn, b_dend, w_dend, b_soma)
    nc = get_nc()
    res = run_bass_kernel_spmd(nc, in_maps, list(range(NCORES)))
    return assemble_output(res.results)


# revision 5
# speedup vs baseline: 1.1265x; 1.0569x over previous
"""Trainium2 Bass kernel for nn_Camada_33612414059004.

Computes, for x:[B,N,D,S], M:[N,N], w_syn:[N,D,S], b_dend:[N,D],
w_dend:[N,D], b_soma:[N]:

    xm    = einsum('bids,oi->bods', x, M)
    dend  = tanh(einsum('bnds,nds->bnd', xm, w_syn) + b_dend)
    soma  = einsum('bnd,nd->bn', dend, w_dend) + b_soma
    out   = sigmoid(soma)                                  # [B, N]

Sharding: data-parallel over batch across 8 NeuronCores (B=64 -> 8/core),
zero cross-core communication.  Per core the dominant work is the
connectivity matmul  M[o,i] @ x[i, (b,d,s)]  ([1024x1024]x[1024x1024]).

Key speed tricks vs the bf16 baseline:
 - fp8(e4m3) matmul operands with perf_mode=DoubleRow: M is an exact 0/1
   matrix (fp8-lossless) and x quantization costs ~0.5% final rel-err
   (tanh/sigmoid compress it).  DoubleRow packs 2 fp8 weights per PE
   cell -> 256-deep contraction per matmul, ~2x PE throughput, and the
   input DMA bytes halve (2MB -> 1MB per tensor per core).
 - 4 contraction pair-steps of 256 input-neurons; 32 full-width matmuls
   (moving free dim 1024 = the whole per-o-tile output, 2 PSUM banks).
 - Postprocess tuned to measured engine rates: every PSUM read (the
   w_syn drain-multiply) on DVE; s-reduces on DVE in the 2x all-SBUF
   mode, batched 4 tiles per instruction; bias/tanh/soma/sigmoid batched
   per tile-group (ACT activations, GpSimd elementwise+trees, which are
   otherwise idle); b_soma folded in before the sigmoid so sigmoids
   batch across tiles.
 - Last o-tile runs half-width matmuls and two independent all-DVE
   latency chains (per batch half), h0 postprocess overlapping h1
   matmuls.
"""

import numpy as np
import ml_dtypes
from contextlib import ExitStack

import concourse.bass as bass
import concourse.mybir as mybir
import concourse.tile as tile

B, N, D, S = 64, 1024, 8, 16
NCORES = 8
BC = B // NCORES          # batches per core = 8
DS = D * S                # 128
P = 128                   # SBUF partitions
KP = 4                    # contraction pair-steps (256 input neurons each)
OT = N // P               # 8 output-neuron tiles
FH = 512                  # one fp32 PSUM bank of moving free dim
FW = 2 * FH               # full-width moving free dim (2 banks)
BD = BC * D               # 64
GRP = 4                   # o-tiles in the pair-outer leading group
B0, W1, B1 = 0, OT * D, 2 * OT * D
SPC = 2 * OT * D + OT     # small params cols = 136

F32 = mybir.dt.float32
BF16 = mybir.dt.bfloat16
F8 = mybir.dt.float8e4

_NC_CACHE = {}


def legalize_waits(nc, max_attached=1):
    """Split multi-semaphore waits onto preceding same-engine NOPs.

    The walrus build in this environment accepts at most one sync-wait
    command per instruction (setupSyncWait: "Too many sync wait commands"),
    but Tile attaches one wait per out-of-date engine clock.  An engine is
    in-order, so hoisting the extra waits onto NOPs immediately before the
    instruction is semantics-preserving.
    """
    nid = 0
    for f in nc.m.functions:
        for blk in f.blocks:
            new = []
            changed = False
            for inst in blk.instructions:
                si = inst.sync_info
                if si is not None and si.on_wait and len(si.on_wait) > max_attached:
                    waits = list(si.on_wait)
                    for w in waits[:-max_attached]:
                        nid += 1
                        nop = mybir.InstNoOp(name=f"WSPLIT-{nid}", ins=[], outs=[])
                        nop.engine = inst.engine
                        nop.sync_info = mybir.SyncInfo(on_wait=[w], on_update=[])
                        new.append(nop)
                    inst.sync_info = mybir.SyncInfo(
                        on_wait=waits[-max_attached:], on_update=list(si.on_update)
                    )
                    changed = True
                new.append(inst)
            if changed:
                blk.instructions = new
    return nc


def build_nc(legalize=True):
    """Build the single-core Bass program (SPMD: same program on all cores)."""
    nc = bass.Bass()
    # mt cols: (o-tile t, pair-member j, o-within-tile) so per-o-tile lhsT
    # slices and the tile-0-first DMA split are both contiguous.
    mt = nc.declare_dram_parameter("mt", [KP * P, OT * 2 * P], F8, isOutput=False)
    # xc cols: (pair-member j, (b, d, s)) -> full-width rhs is a clean 3D AP.
    xc = nc.declare_dram_parameter("xc", [KP * P, 2 * FW], F8, isOutput=False)
    wsyn = nc.declare_dram_parameter("wsyn", [P, OT * DS], BF16, isOutput=False)
    smallp = nc.declare_dram_parameter("smallp", [P, SPC], F32, isOutput=False)
    out = nc.declare_dram_parameter("out", [P, OT * BC], F32, isOutput=True)

    AF = mybir.ActivationFunctionType
    AX = mybir.AxisListType
    OP = mybir.AluOpType
    DR = mybir.MatmulPerfMode.DoubleRow

    with tile.TileContext(nc) as tc, ExitStack() as ctx:
        wpool = ctx.enter_context(tc.tile_pool(name="weights", bufs=1))
        xpool = ctx.enter_context(tc.tile_pool(name="xin", bufs=1))
        pspool = ctx.enter_context(tc.tile_pool(name="ps", bufs=4, space="PSUM"))
        smpool = ctx.enter_context(tc.tile_pool(name="smp", bufs=3))

        # --- PE pre-warm: dummy matmuls on memset scratch while the first
        # input chunk is in flight; the HAM clock-gate needs ~3.4us of PE
        # activity before it lifts the PE from 1.2 to 2.4 GHz, so start
        # that window as early as possible with a minimal memset. ---
        warm_sb = wpool.tile([P, P], BF16, tag="warm", name="warm_sb")
        nc.gpsimd.memset(warm_sb[:], 0.0)
        warm_ps = pspool.tile([P, FW], F32, tag="ps", name="warm_ps")
        for _ in range(7):
            nc.tensor.matmul(
                warm_ps[:, 0:P], lhsT=warm_sb[:], rhs=warm_sb[:],
                start=True, stop=True,
            )

        # --- input DMAs: x on Sync HWDGE, mt on Scalar HWDGE (parallel
        # issue).  Pair 0 is split so the very first matmul (o-tile 0)
        # can start after ~160KB instead of 512KB. ---
        x_tiles, mt_tiles = [], []
        xdmas = []
        for p in range(KP):
            xt = xpool.tile([P, 2 * FW], F8, tag=f"x{p}", name=f"x{p}")
            mtp = xpool.tile([P, OT * 2 * P], F8, tag=f"m{p}", name=f"m{p}")
            r0, r1 = p * P, (p + 1) * P
            if p == 0:
                xdmas.append(nc.sync.dma_start(xt[:, 0:FW], xc[r0:r1, 0:FW]))
                xdmas.append(nc.sync.dma_start(xt[:, FW:], xc[r0:r1, FW:]))
                nc.scalar.dma_start(mtp[:, 0:2 * P], mt[r0:r1, 0:2 * P])
                nc.scalar.dma_start(mtp[:, 2 * P:], mt[r0:r1, 2 * P:])
            else:
                xdmas.append(nc.sync.dma_start(xt[:], xc[r0:r1, :]))
                nc.scalar.dma_start(mtp[:], mt[r0:r1, :])
            x_tiles.append(xt)
            mt_tiles.append(mtp)

        # Per-neuron params ride behind the matmul-critical stream: first
        # needed when the leading postprocess starts (~after pair 3).
        from bass_rust import add_dep_helper
        smallp_sb = wpool.tile([P, SPC], F32, tag="smallp", name="smallp_sb")
        sdma = nc.gpsimd.dma_start(smallp_sb[:], smallp[:, :])
        add_dep_helper(sdma.ins, xdmas[1].ins, sync=True,
                       reason="small params after first x pair")
        wsyn_sb = wpool.tile([P, OT * DS], BF16, tag="wsyn", name="wsyn_sb")
        wdma = nc.gpsimd.dma_start(wsyn_sb[:], wsyn[:, :])
        add_dep_helper(wdma.ins, xdmas[2].ins, sync=True,
                       reason="w_syn after second x pair")

        prod_all = wpool.tile([P, OT * BC * DS], F32, tag="prod", name="prod_all")
        dp_all = wpool.tile([P, OT * BD], F32, tag="dp", name="dp_all")
        dend_all = wpool.tile([P, OT * BD], F32, tag="dend", name="dend_all")
        soma_all = wpool.tile([P, OT * BC], F32, tag="soma", name="soma_all")
        out_sb = wpool.tile([P, OT * BC], F32, tag="out", name="out_sb")

        def lhsT(t, p):
            return (mt_tiles[p][:, t * 2 * P:(t + 1) * 2 * P]
                    .rearrange("p (j o) -> p j o", j=2))

        def mm_full(pst, t, p):
            for h in range(2):
                mm_half(pst, t, p, h)

        def mm_half(pst, t, p, h):
            nc.tensor.matmul(
                pst[:, h * FH:(h + 1) * FH], lhsT=lhsT(t, p),
                rhs=x_tiles[p][:].rearrange("p (j c) -> p j c", j=2)
                [:, :, h * FH:(h + 1) * FH],
                start=(p == 0), stop=(p == KP - 1), perf_mode=DR,
            )

        def drain(t, pst, b=BC, h=0):
            # prod[o, b, (d,s)] = psum * w_syn (broadcast over b), one wide
            # DVE op straight from the 2-bank PSUM tile.
            pr = prod_all[:, t * BC * DS + h * (BC // 2) * DS:
                          t * BC * DS + h * (BC // 2) * DS + b * DS]
            nc.vector.tensor_mul(
                pr.rearrange("p (b q) -> p b q", b=b),
                pst.rearrange("p (b q) -> p b q", b=b),
                wsyn_sb[:, t * DS:(t + 1) * DS].unsqueeze(1)
                .broadcast_to([P, b, DS]),
            )
            return pr

        def sred(t_lo, n_bd):
            # 2x all-SBUF DVE mode; batches n_bd*16 input cols.
            nc.vector.tensor_reduce(
                dp_all[:, t_lo * BD:t_lo * BD + n_bd],
                prod_all[:, t_lo * BC * DS:t_lo * BC * DS + n_bd * S]
                .rearrange("p (bd s) -> p bd s", s=S),
                axis=AX.X, op=OP.add,
            )

        def bias_gps(t_lo, nt):
            # dp += b_dend, nt tiles in one GpSimd op (broadcast over b).
            nc.gpsimd.tensor_add(
                dp_all[:, t_lo * BD:(t_lo + nt) * BD]
                .rearrange("p (t b d) -> p t b d", b=BC, d=D),
                dp_all[:, t_lo * BD:(t_lo + nt) * BD]
                .rearrange("p (t b d) -> p t b d", b=BC, d=D),
                smallp_sb[:, B0 + t_lo * D:B0 + (t_lo + nt) * D]
                .rearrange("p (t d) -> p t d", d=D).unsqueeze(2)
                .broadcast_to([P, nt, BC, D]),
            )

        def tanh_batch(t_lo, t_hi):
            nc.scalar.activation(
                dend_all[:, t_lo * BD:t_hi * BD],
                dp_all[:, t_lo * BD:t_hi * BD], AF.Tanh)

        def soma_gps(t_lo, nt):
            # sp = dend * w_dend; pairwise d-tree; + b_soma — all batched
            # over nt tiles on the (otherwise idle) GpSimd engine.
            sp = smpool.tile([P, nt * BD], F32, tag="sp", name=f"sp{t_lo}")
            nc.gpsimd.tensor_mul(
                sp[:].rearrange("p (t b d) -> p t b d", b=BC, d=D),
                dend_all[:, t_lo * BD:(t_lo + nt) * BD]
                .rearrange("p (t b d) -> p t b d", b=BC, d=D),
                smallp_sb[:, W1 + t_lo * D:W1 + (t_lo + nt) * D]
                .rearrange("p (t d) -> p t d", d=D).unsqueeze(2)
                .broadcast_to([P, nt, BC, D]),
            )
            spv = sp[:].rearrange("p (tb d) -> p tb d", d=D)
            r1 = smpool.tile([P, nt * BC * 4], F32, tag="r1", name=f"r1{t_lo}")
            nc.gpsimd.tensor_add(
                r1[:].rearrange("p (tb d) -> p tb d", d=4),
                spv[:, :, 0:4], spv[:, :, 4:8])
            r1v = r1[:].rearrange("p (tb d) -> p tb d", d=4)
            r2 = smpool.tile([P, nt * BC * 2], F32, tag="r2", name=f"r2{t_lo}")
            nc.gpsimd.tensor_add(
                r2[:].rearrange("p (tb d) -> p tb d", d=2),
                r1v[:, :, 0:2], r1v[:, :, 2:4])
            r2v = r2[:].rearrange("p (tb d) -> p tb d", d=2)
            r3 = smpool.tile([P, nt * BC], F32, tag="r3", name=f"r3{t_lo}")
            nc.gpsimd.tensor_add(
                r3[:].unsqueeze(2), r2v[:, :, 0:1], r2v[:, :, 1:2])
            # b_soma folded here so the sigmoid can batch tiles with
            # different b_soma columns.
            nc.gpsimd.tensor_add(
                soma_all[:, t_lo * BC:(t_lo + nt) * BC]
                .rearrange("p (t b) -> p t b", b=BC),
                r3[:].rearrange("p (t b) -> p t b", b=BC),
                smallp_sb[:, B1 + t_lo:B1 + t_lo + nt].unsqueeze(2)
                .broadcast_to([P, nt, BC]),
            )

        def sigmoid_batch(t_lo, t_hi):
            nc.scalar.activation(
                out_sb[:, t_lo * BC:t_hi * BC],
                soma_all[:, t_lo * BC:t_hi * BC], AF.Sigmoid)

        # ---- matmuls ----
        pst = {}
        for t in range(GRP):
            pst[t] = pspool.tile([P, FW], F32, tag="ps", name=f"ps{t}")
        # Leading group: pair-outer over o-tiles 0-3 rides the DMA stream.
        for p in range(KP):
            for t in range(GRP):
                mm_full(pst[t], t, p)
        # Trailing o-tiles: pair-inner, one at a time (banks freed by the
        # leading drains); last tile half-outer so h0's postprocess chain
        # overlaps h1's matmuls.
        for t in range(GRP, OT):
            pst[t] = pspool.tile([P, FW], F32, tag="ps", name=f"ps{t}")
            if t < OT - 1:
                for p in range(KP):
                    mm_full(pst[t], t, p)
            else:
                for h in range(2):
                    for p in range(KP):
                        mm_half(pst[t], t, p, h)

        # ---- postprocess ----
        # Leading tiles 0-3 (stops cluster right after pair 3): per-tile
        # DVE drains (each also frees the PSUM banks for a trailing tile),
        # then group-batched everything else.
        for t in range(4):
            drain(t, pst[t][:])
        sred(0, 4 * BD)
        bias_gps(0, 4)
        tanh_batch(0, 4)
        soma_gps(0, 4)
        sigmoid_batch(0, 4)
        # Trailing tiles 4-5.
        drain(4, pst[4][:])
        sred(4, BD)
        drain(5, pst[5][:])
        sred(5, BD)
        bias_gps(4, 2)
        tanh_batch(4, 6)
        soma_gps(4, 2)
        sigmoid_batch(4, 6)
        nc.scalar.dma_start(out[:, 0:6 * BC], out_sb[:, 0:6 * BC])
        # Tile 6.
        drain(6, pst[6][:])
        sred(6, BD)
        bias_gps(6, 1)
        tanh_batch(6, 7)
        soma_gps(6, 1)
        sigmoid_batch(6, 7)
        nc.scalar.dma_start(out[:, 6 * BC:7 * BC], out_sb[:, 6 * BC:7 * BC])
        # Last tile: two independent all-DVE latency chains (one per
        # PSUM half = batch half).
        t = OT - 1
        HB = BC // 2          # 4 batches per half
        for h in range(2):
            drain(t, pst[t][:, h * FH:(h + 1) * FH], b=HB, h=h)
            dps = dp_all[:, t * BD + h * HB * D:t * BD + (h + 1) * HB * D]
            nc.vector.tensor_reduce(
                dps,
                prod_all[:, t * BC * DS + h * HB * DS:
                         t * BC * DS + (h + 1) * HB * DS]
                .rearrange("p (bd s) -> p bd s", s=S),
                axis=AX.X, op=OP.add)
            nc.vector.tensor_add(
                dps.rearrange("p (b d) -> p b d", d=D),
                dps.rearrange("p (b d) -> p b d", d=D),
                smallp_sb[:, B0 + t * D:B0 + (t + 1) * D].unsqueeze(1)
                .broadcast_to([P, HB, D]),
            )
            dnds = dend_all[:, t * BD + h * HB * D:t * BD + (h + 1) * HB * D]
            nc.scalar.activation(dnds, dps, AF.Tanh)
            sp7 = smpool.tile([P, HB * D], F32, tag="sp7", name=f"sp7{h}")
            nc.vector.tensor_mul(
                sp7[:].rearrange("p (b d) -> p b d", d=D),
                dnds.rearrange("p (b d) -> p b d", d=D),
                smallp_sb[:, W1 + t * D:W1 + (t + 1) * D].unsqueeze(1)
                .broadcast_to([P, HB, D]),
            )
            smp7 = smpool.tile([P, HB], F32, tag="smp7", name=f"smp7{h}")
            nc.vector.tensor_reduce(
                smp7[:], sp7[:].rearrange("p (b d) -> p b d", d=D),
                axis=AX.X, op=OP.add)
            sms = soma_all[:, t * BC + h * HB:t * BC + (h + 1) * HB]
            nc.vector.tensor_add(
                sms, smp7[:],
                smallp_sb[:, B1 + t:B1 + t + 1].broadcast_to([P, HB]))
            nc.scalar.activation(
                out_sb[:, t * BC + h * HB:t * BC + (h + 1) * HB],
                sms, AF.Sigmoid)

        nc.scalar.dma_start(out[:, 7 * BC:], out_sb[:, 7 * BC:])

    if legalize:
        legalize_waits(nc)
    return nc


def get_nc():
    if "nc" not in _NC_CACHE:
        _NC_CACHE["nc"] = build_nc()
    return _NC_CACHE["nc"]


def pack_static(matriz_conexao, w_syn, b_dend, w_dend, b_soma):
    """Pack the batch-independent operands (shared by all cores)."""
    # mt rows (pair p, r), cols (t, j, o): lhsT[r, j, o] = M[t*128+o, (2p+j)*128+r]
    mtT = np.ascontiguousarray(np.asarray(matriz_conexao, np.float32).T)  # [i, o]
    mt_np = (mtT.reshape(KP, 2, P, OT, P)        # [p, j, r, t, o]
             .transpose(0, 2, 3, 1, 4)           # [p, r, t, j, o]
             .reshape(KP * P, OT * 2 * P)
             .astype(ml_dtypes.float8_e4m3))
    ws = (np.asarray(w_syn, np.float32).reshape(OT, P, DS).transpose(1, 0, 2)
          .reshape(P, OT * DS).astype(ml_dtypes.bfloat16))
    bd = np.asarray(b_dend, np.float32).reshape(OT, P, D).transpose(1, 0, 2).reshape(P, OT * D)
    wd = np.asarray(w_dend, np.float32).reshape(OT, P, D).transpose(1, 0, 2).reshape(P, OT * D)
    bs = np.asarray(b_soma, np.float32).reshape(OT, P).T
    smallp_np = np.ascontiguousarray(
        np.concatenate([bd, wd, bs], axis=1).astype(np.float32))
    return mt_np, np.ascontiguousarray(ws), smallp_np


def prepare_in_maps(x, matriz_conexao, w_syn, b_dend, w_dend, b_soma):
    mt_np, ws_np, smallp_np = pack_static(matriz_conexao, w_syn, b_dend, w_dend, b_soma)
    x = np.asarray(x, np.float32)
    xq = x.astype(ml_dtypes.float8_e4m3)
    # xt[i, b, (d,s)] then per core rows (pair p, r), cols (j, b, d, s)
    xt = np.ascontiguousarray(xq.transpose(1, 0, 2, 3).reshape(N, B, DS))
    in_maps = []
    for c in range(NCORES):
        xcor = xt[:, c * BC:(c + 1) * BC, :]          # [N, 8, 128]
        xc_np = np.ascontiguousarray(
            xcor.reshape(KP, 2, P, BC * DS)            # [p, j, r, c]
            .transpose(0, 2, 1, 3)                     # [p, r, j, c]
            .reshape(KP * P, 2 * FW))
        in_maps.append({"mt": mt_np, "xc": xc_np,
                        "wsyn": ws_np, "smallp": smallp_np})
    return in_maps


def assemble_output(results):
    outs = []
    for c in range(NCORES):
        oc = np.asarray(results[c]["out"])          # [P, (t, b)]
        outs.append(oc.reshape(P, OT, BC).transpose(2, 1, 0).reshape(BC, N))
    return np.ascontiguousarray(np.concatenate(outs, axis=0).astype(np.float32))


def kernel(x, matriz_conexao, w_syn, b_dend, w_dend, b_soma):
    from concourse.bass_utils import run_bass_kernel_spmd
    in_maps = prepare_in_maps(x, matriz_conexao, w_syn, b_dend, w_dend, b_soma)
    nc = get_nc()
    res = run_bass_kernel_spmd(nc, in_maps, list(range(NCORES)))
    return assemble_output(res.results)
